# revision 46
# baseline (speedup 1.0000x reference)
"""GQA causal attention block (RoPE, 32 Q heads / 8 KV heads, S=2048, D=4096)
distributed tensor-parallel over heads across 8 TRN2 NeuronCores.

Per core c:
  - 4 query heads (wq cols 512c:512c+512), 1 KV head (wk/wv cols 128c:128c+128)
  - projections computed transposed (qT/kT/vT [hd, seq]) from host-transposed
    xT, weights as stationary operands, bf16 matmuls with f32 PSUM accum
  - RoPE applied with host tables; the half-rotation uses a PE permutation
    matmul (cross-partition moves are impossible on DVE)
  - attention in S^T layout [j, i]; heads processed in pairs with a one-jt
    software skew: the PE computes scores for tile jt while the ACT exp of
    tile jt-1 is consumed by the PV/rowsum matmuls, so the exp latency is
    fully hidden behind double-buffered score banks
  - per-pair O^T accumulates in a single bf16 PSUM bank ([128,2,512]), which
    frees banks for score double-buffering; causal masks are multiplicative
    0/1 DVE ops after the exp; row sums accumulate via col-tiled M=1 matmuls
  - 1/l computed as exp(-ln(l)) on ACT, broadcast across partitions with a
    DRAM-bounce stride-0 DMA, normalization on DVE into an AllGather staging
    tile; one DMA + AllGather per 512-seq group overlaps with compute
  - output projection computed transposed (outT = wo_c^T @ Y^T) from
    SBUF-resident wo, bf16 stores; host casts/transposes each shard back
Host gathers by concatenating the 8 (transposed) column shards.

DMA issue is spread across queues (weights on the GpSimd SWDGE queue, bulk
activations on Sync, stores on Scalar) and batched 4 tiles per descriptor
set so no single sequencer saturates. PSUM banks are laid out as four 2KB
regions (r0-r3) and two 4KB regions (r45, r67) shared across phases.
"""

import numpy as np
import ml_dtypes

import concourse.bass as bass
import concourse.mybir as mybir
import concourse.tile as tile
from concourse.bass_utils import run_bass_kernel_spmd

bf16 = mybir.dt.bfloat16
f32 = mybir.dt.float32

NCORES = 8
S = 2048
DIM = 4096
HD = 128
NH = 32
QH = NH // NCORES          # 4 query heads per core
QW = QH * HD               # 512 wq cols per core
ROPE_BASE = 10000.0
SCALE = float(HD) ** -0.5
NSEQ = S // 512            # 4 seq chunks
KT = DIM // 128            # 32 contraction tiles

_CACHE = {}


def _legalize_waits(nc, allowed_default=1):
    """This walrus build rejects instructions carrying more inline sync waits
    than the opcode template allows (0 for Drain, 1 elsewhere). Spill excess
    waits onto standalone EventSemaphore instructions inserted immediately
    before, on the same engine (engine order preserves semantics)."""
    for f in nc.m.functions:
        for bb in f.blocks:
            out = []
            for ins in bb.instructions:
                tname = type(ins).__name__
                si = getattr(ins, "sync_info", None)
                waits = list(si.on_wait) if (si is not None and si.on_wait) else []
                if tname == "InstEventSemaphore":
                    allowed = len(waits)
                elif tname == "InstDrain":
                    allowed = 0
                else:
                    allowed = allowed_default
                if len(waits) > allowed:
                    spill, keep = waits[allowed:], waits[:allowed]
                    for i, w in enumerate(spill):
                        ev = mybir.InstEventSemaphore(
                            name=f"{ins.name}_wfix{i}",
                            engine=ins.engine, ins=[], outs=[],
                        )
                        ev.sync_info = mybir.SyncInfo(on_wait=[w], on_update=[])
                        out.append(ev)
                    si.on_wait = keep
                out.append(ins)
            bb.instructions[:] = out


def _build_nc():
    nc = bass.Bass(num_devices=NCORES)

    # weights and x arrive host-pre-tiled in SBUF layout [partition, ktile,
    # cols] so every DMA line is 1-8KB contiguous (few fat descriptors)
    xT = nc.declare_dram_parameter("xT", [NSEQ, 128, KT, 512], bf16,
                                   isOutput=False)
    wq = nc.declare_dram_parameter("wq", [128, KT, QW], bf16, isOutput=False)
    wk = nc.declare_dram_parameter("wk", [128, KT, HD], bf16, isOutput=False)
    wv = nc.declare_dram_parameter("wv", [128, KT, HD], bf16, isOutput=False)
    wo = nc.declare_dram_parameter("wo", [128, KT, QW], bf16, isOutput=False)
    cosT = nc.declare_dram_parameter("cosT", [HD, S], f32, isOutput=False)
    sinT = nc.declare_dram_parameter("sinT", [HD, S], f32, isOutput=False)
    maskB = nc.declare_dram_parameter("maskB", [4, 128, 512], bf16, isOutput=False)
    perm = nc.declare_dram_parameter("perm", [128, 128], bf16, isOutput=False)
    ident = nc.declare_dram_parameter("ident", [128, 128], bf16, isOutput=False)
    outT = nc.declare_dram_parameter("outT", [QW, S], bf16, isOutput=True)

    ag_in = nc.dram_tensor("ag_in", [4, QW, 512], bf16)
    ag_outs = [
        nc.dram_tensor(f"ag_out{g}", [NCORES, QW, 512], bf16, addr_space="Shared")
        for g in range(4)
    ]

    maskr = maskB.rearrange("a p m -> p a m")
    agi_r = ag_in.rearrange("g (h p) m -> g p h m", p=128)
    ago_r = [a.rearrange("c (d p) m -> c p d m", p=128) for a in ag_outs]
    outT_r = outT.rearrange("(a p) m -> p a m", p=128)

    with tile.TileContext(nc) as tc:
        with (
            tc.tile_pool(name="const", bufs=1) as constp,
            tc.tile_pool(name="acts", bufs=1) as acts,
            tc.tile_pool(name="xin", bufs=3) as xin,
            tc.tile_pool(name="rope", bufs=2) as rope,
            tc.tile_pool(name="pt", bufs=3) as ptp,
            tc.tile_pool(name="epi", bufs=1) as epi,
            tc.tile_pool(name="cproj", bufs=4) as cproj,
            tc.tile_pool(name="psum", bufs=1, space="PSUM") as psum,
        ):
            # PSUM layout: r0-r3 are 2KB regions (banks 0-3), r45/r67 are 4KB
            # regions (banks 4-5 / 6-7). Shapes/dtypes vary per phase; the
            # byte footprint per tag is constant.
            def preg(tag, shape, dtype=f32, name="ps"):
                return psum.tile(list(shape), dtype, tag=tag,
                                 name=f"{name}_{tag}", bufs=1)

            # small constants (cheap DMAs on sync)
            perm_sb = constp.tile([128, 128], bf16)
            nc.sync.dma_start(perm_sb[:], perm[:])
            ident_sb = constp.tile([128, 128], bf16)
            nc.sync.dma_start(ident_sb[:], ident[:])
            ones_sb = constp.tile([128, 1], bf16)
            nc.vector.memset(ones_sb[:], 1.0)
            ones_row = constp.tile([33, 128], bf16)
            nc.vector.memset(ones_row[:], 1.0)

            # persistent SBUF residents
            wq_sb = constp.tile([128, KT, QW], bf16)
            wk_sb = constp.tile([128, KT, HD], bf16)
            wv_sb = constp.tile([128, KT, HD], bf16)
            wo_sb = constp.tile([128, KT, QW], bf16)
            cos_sb = constp.tile([HD, S], f32)
            sin_sb = constp.tile([HD, S], f32)
            mask_sb = constp.tile([128, 4, 512], bf16)

            # DMA-queue arbitration is near-exhaustive per queue, so a bulk
            # flood on any queue starves the others. Chunk 0's x tiles and wq
            # share the gpsimd queue in exact consumption order (self-paced);
            # wk/wv/cos/sin/mask/wo stream on scalar; sync only carries the
            # later chunks' x tiles plus phase B/C traffic.

            # activations that live through attention
            qTr = acts.tile([128, QH, S], bf16)      # 4 head tiles [hd, seq]
            kTr = acts.tile([128, S], bf16)
            v_sb = acts.tile([128, S], bf16)         # 16 [seq,hd] tiles at jt*128

            # ---- phase A: projections + rope ----
            WQC = {0: (2, 8), 1: (8, 16), 3: (16, 24), 5: (24, 32)}
            for n in range(NSEQ):
                sl = bass.ts(n, 512)
                x4s = []
                for t in range(8):
                    x4 = xin.tile([128, 4, 512], bf16, tag="x")
                    if n == 0:
                        if t == 0:
                            nc.gpsimd.dma_start(wq_sb[:, 0:2], wq[:, 0:2])
                            nc.scalar.dma_start(wk_sb[:, 0:2], wk[:, 0:2])
                            nc.scalar.dma_start(wv_sb[:, 0:2], wv[:, 0:2])
                            nc.gpsimd.dma_start(x4[:, 0:1], xT[n][:, 0:1])
                            nc.gpsimd.dma_start(x4[:, 1:4], xT[n][:, 1:4])
                        else:
                            nc.gpsimd.dma_start(x4[:], xT[n][:, bass.ts(t, 4)])
                        if t in WQC:
                            a, b = WQC[t]
                            nc.gpsimd.dma_start(wq_sb[:, a:b], wq[:, a:b])
                    else:
                        nc.sync.dma_start(x4[:], xT[n][:, bass.ts(t, 4)])
                    x4s.append(x4)
                if n == 0:
                    for i in range(4):
                        lo = slice(8 * i + (2 if i == 0 else 0), 8 * i + 8)
                        nc.scalar.dma_start(wk_sb[:, lo], wk[:, lo])
                        nc.scalar.dma_start(wv_sb[:, lo], wv[:, lo])
                    nc.scalar.dma_start(cos_sb[:], cosT[:])
                    nc.scalar.dma_start(sin_sb[:], sinT[:])
                    nc.scalar.dma_start(mask_sb[:], maskr[:])
                    for i in range(4):
                        nc.scalar.dma_start(wo_sb[:, bass.ts(i, 8)],
                                            wo[:, bass.ts(i, 8)])
                q_ps = [preg(f"r{m}", (128, 512), name="q") for m in range(QH)]
                kv3 = preg("s456", (128, 3, 512), name="kv")
                k_ps, vT_ps = kv3[:, 0], kv3[:, 1]
                for k in range(KT):
                    x_sb = x4s[k // 4][:, k % 4]
                    st, sp = (k == 0), (k == KT - 1)
                    for m in range(QH):
                        nc.tensor.matmul(q_ps[m][:], wq_sb[:, k, bass.ts(m, 128)],
                                         x_sb, start=st, stop=sp)
                    nc.tensor.matmul(k_ps, wk_sb[:, k], x_sb, start=st, stop=sp)
                    nc.tensor.matmul(vT_ps, wv_sb[:, k], x_sb, start=st, stop=sp)

                # rope: free the accumulation banks first (ACT copy + DVE
                # cos-mul per output), then the rotation matmuls and combines
                t_bfs, t1s = [], []
                for idx in range(QH + 1):
                    src = q_ps[idx][:] if idx < QH else k_ps
                    t_bf = rope.tile([128, 512], bf16, tag=f"tbf{idx}",
                                     name=f"tbf{idx}", bufs=1)
                    nc.scalar.copy(t_bf[:], src)
                    t1 = rope.tile([128, 512], bf16, tag=f"t1_{idx}",
                                   name=f"t1_{idx}", bufs=1)
                    nc.vector.tensor_mul(t1[:], src, cos_sb[:, sl])
                    t_bfs.append(t_bf)
                    t1s.append(t1)
                for idx in range(QH + 1):
                    dst = qTr[:, idx, sl] if idx < QH else kTr[:, sl]
                    sw_ps = (kv3[:, 2] if idx % 2 == 0
                             else preg("r7", (128, 512), name="sw")[:])
                    nc.tensor.matmul(sw_ps, perm_sb[:], t_bfs[idx][:],
                                     start=True, stop=True)
                    t2 = rope.tile([128, 512], bf16, tag=f"t2_{idx % 2}",
                                   name=f"t2_{idx % 2}", bufs=1)
                    nc.vector.tensor_mul(t2[:], sw_ps, sin_sb[:, sl])
                    nc.vector.tensor_add(dst, t1s[idx][:], t2[:])

                # v: copy vT chunk, transpose 128-blocks into [seq, hd] tiles
                v_bf = rope.tile([128, 512], bf16, tag="vbf")
                nc.scalar.copy(v_bf[:], vT_ps)
                for t in range(4):
                    vt_ps = preg("s456" if t % 2 == 0 else "r7",
                                 (128, 128), dtype=bf16, name="vt")
                    nc.tensor.transpose(vt_ps[:], v_bf[:, bass.ts(t, 128)],
                                        ident_sb[:])
                    nc.vector.tensor_copy(out=v_sb[:, bass.ts(4 * n + t, 128)],
                                          in_=vt_ps[:])
                if n == NSEQ - 1:
                    # keep the PE warm through the final rope drain so HAM
                    # doesn't re-throttle before attention starts
                    for dmy in range(6):
                        dps = preg(f"r{dmy % 2}", (128, 512), name="warm")
                        nc.tensor.matmul(dps[:], perm_sb[:], kTr[:, 0:512],
                                         start=True, stop=True)

            # ---- phase B: attention, S^T layout, head pairs, one-jt skew ----
            for g in range(4):
                isl = bass.ts(g, 512)
                njt = 4 * g + 4
                ag_sb = epi.tile([128, QH, 512], bf16, tag="agsb", bufs=2)
                # l for all four heads shares bank r7 (rows 0/32/64/96 via
                # col-group tile_position); matmul start=True would clear the
                # whole bank's has_written bits and corrupt the other rows,
                # so the bank is zeroed once and every l matmul accumulates
                # with start=False (add-to-zero or overwrite, both correct)
                l_ps = preg("r7", (128, 512), name="l")
                nc.vector.memset(l_ps[:], 0.0)
                for p in range(2):
                    oT_ps = [preg(f"r{2 * p + hh}", (128, 512), name="oT")
                             for hh in range(2)]
                    st_big = preg("s456", (128, 3, 512), name="st")
                    prev = None
                    for jt in range(njt):
                        ksl = bass.ts(jt, 128)
                        pt = ptp.tile([128, 2, 512], bf16, tag="pt")
                        a0, a1 = (2 * jt) % 3, (2 * jt + 1) % 3
                        for hh, aa in ((0, a0), (1, a1)):
                            nc.tensor.matmul(st_big[:, aa], kTr[:, ksl],
                                             qTr[:, 2 * p + hh, isl],
                                             start=True, stop=True)
                        if a1 == a0 + 1:
                            nc.scalar.activation(
                                pt[:], st_big[:, a0:a0 + 2],
                                mybir.ActivationFunctionType.Exp, scale=SCALE)
                        else:
                            for hh, aa in ((0, a0), (1, a1)):
                                nc.scalar.activation(
                                    pt[:, hh], st_big[:, aa],
                                    mybir.ActivationFunctionType.Exp,
                                    scale=SCALE)
                        r = jt - 4 * g
                        if r >= 0:
                            ptm = ptp.tile([128, 2, 512], bf16, tag="ptm")
                            for hh in range(2):
                                nc.vector.tensor_mul(ptm[:, hh], pt[:, hh],
                                                     mask_sb[:, r])
                            pt = ptm

                        def consume(cjt, cpt):
                            cksl = bass.ts(cjt, 128)
                            for hh in range(2):
                                nc.tensor.matmul(oT_ps[hh][:], v_sb[:, cksl],
                                                 cpt[:, hh], start=(cjt == 0),
                                                 stop=(cjt == njt - 1))
                            for hh in range(2):
                                row = 64 * p + 32 * hh
                                nc.tensor.matmul(
                                    l_ps[bass.ds(row, 1), :], ones_sb[:],
                                    cpt[:, hh], start=False,
                                    stop=(cjt == njt - 1),
                                    tile_position=(0, row),
                                    skip_group_check=True)

                        if prev is not None:
                            consume(*prev)
                        prev = (jt, pt)
                    consume(*prev)
                    # epilogue: 1/l = exp(-ln(l)) on ACT, partition-broadcast
                    # via DRAM-bounce stride-0 DMA, normalize on DVE
                    ln_sb = epi.tile([33, 512], f32, tag=f"ln{p}")
                    nc.scalar.activation(ln_sb[:], l_ps[64 * p:64 * p + 33, :],
                                         mybir.ActivationFunctionType.Ln)
                    linv_sb = epi.tile([33, 512], bf16, tag=f"linv{p}")
                    nc.scalar.activation(linv_sb[:], ln_sb[:],
                                         mybir.ActivationFunctionType.Exp,
                                         scale=-1.0)
                    # partition-broadcast 1/l rows with a K=1 matmul
                    # (ones_row^T @ linv_row), DVE-copy to SBUF, normalize
                    for hh in range(2):
                        lb_ps = st_big[:, 2]
                        nc.tensor.matmul(lb_ps, ones_row[bass.ds(32 * hh, 1), :],
                                         linv_sb[bass.ds(32 * hh, 1), :],
                                         start=True, stop=True)
                        lb_sb = epi.tile([128, 512], f32, tag=f"lb{hh}")
                        nc.vector.tensor_copy(out=lb_sb[:], in_=lb_ps)
                        nc.vector.tensor_mul(ag_sb[:, 2 * p + hh],
                                             oT_ps[hh][:], lb_sb[:])
                nc.sync.dma_start(agi_r[g], ag_sb[:])
                nc.gpsimd.collective_compute(
                    "AllGather", mybir.AluOpType.bypass,
                    replica_groups=[list(range(NCORES))],
                    ins=[ag_in[g]], outs=[ag_outs[g][:]],
                )

            # ---- phase C: outT = wo_c^T @ Y^T, wo stationary from SBUF ----
            for ns in range(NSEQ):
                nsl = bass.ts(ns, 512)
                y4s = []
                for c in range(NCORES):
                    y4 = cproj.tile([128, 4, 512], bf16, tag="y")
                    nc.sync.dma_start(y4[:], ago_r[ns][c])
                    y4s.append(y4)
                # even ns uses banks 4-7 (free right after phase B's scores),
                # odd ns uses banks 0-3 (freed by the last epilogue muls)
                if ns % 2 == 0:
                    o3 = preg("s456", (128, 3, 512), name="o")
                    o_ps = [o3[:, 0], o3[:, 1], o3[:, 2],
                            preg("r7", (128, 512), name="o")[:]]
                else:
                    o_ps = [preg(f"r{ob}", (128, 512), name="o")[:]
                            for ob in range(QH)]
                for kt in range(KT):
                    y_sb = y4s[kt // 4][:, kt % 4]
                    for ob in range(QH):
                        nc.tensor.matmul(
                            o_ps[ob], wo_sb[:, kt, bass.ts(ob, 128)], y_sb,
                            start=(kt == 0), stop=(kt == KT - 1))
                o_stage = cproj.tile([128, 4, 512], bf16, tag="ostg", bufs=2)
                for ob in range(QH):
                    nc.scalar.copy(o_stage[:, ob], o_ps[ob])
                    if ob % 2 == 1:
                        nc.scalar.dma_start(
                            outT_r[:, ob - 1:ob + 1, nsl],
                            o_stage[:, ob - 1:ob + 1])

    _legalize_waits(nc)
    return nc


def _pretile(w):
    """[DIM, C] -> SBUF layout [128, KT, C] (partition-major k-tiles)."""
    c = w.shape[1]
    return np.ascontiguousarray(
        np.asarray(w).reshape(KT, 128, c).transpose(1, 0, 2)
    ).astype(ml_dtypes.bfloat16)


def _host_inputs(x, wq, wk, wv, wo):
    x = np.asarray(x, dtype=np.float32)
    xT = np.ascontiguousarray(x.reshape(S, DIM).T)          # [DIM, S]
    # pre-tile x per 512-seq chunk: xP[n, p, a, m] = xT[a*128+p, n*512+m]
    xP = np.ascontiguousarray(
        xT.reshape(KT, 128, NSEQ, 512).transpose(2, 1, 0, 3)
    ).astype(ml_dtypes.bfloat16)

    # rope tables in [hd, seq] layout with the sign of sin baked in
    inv_freq = 1.0 / ROPE_BASE ** (np.arange(0, HD, 2, dtype=np.float32) / HD)
    t = np.arange(S, dtype=np.float32)
    freqs = np.outer(inv_freq, t)                       # [64, S]
    cosT = np.concatenate([np.cos(freqs), np.cos(freqs)], 0).astype(np.float32)
    sinT = np.concatenate([-np.sin(freqs), np.sin(freqs)], 0).astype(np.float32)

    # S^T-layout multiplicative diagonal masks:
    # maskB[r][j, i] = 1 if r*128 + j <= i else 0
    j = np.arange(128)[None, :, None]
    i = np.arange(512)[None, None, :]
    r = np.arange(4)[:, None, None]
    maskB = np.where(r * 128 + j <= i, 1.0, 0.0).astype(ml_dtypes.bfloat16)

    perm = np.zeros((128, 128), dtype=np.float32)
    perm[np.arange(128), (np.arange(128) + 64) % 128] = 1.0
    ident = np.eye(128, dtype=np.float32)

    shared = {
        "xT": xP,
        "cosT": cosT,
        "sinT": sinT,
        "maskB": maskB,
        "perm": perm.astype(ml_dtypes.bfloat16),
        "ident": ident.astype(ml_dtypes.bfloat16),
    }
    maps = []
    for c in range(NCORES):
        m = dict(shared)
        m["wq"] = _pretile(wq[:, c * QW:(c + 1) * QW])
        m["wk"] = _pretile(wk[:, c * HD:(c + 1) * HD])
        m["wv"] = _pretile(wv[:, c * HD:(c + 1) * HD])
        m["wo"] = _pretile(wo[:, c * QW:(c + 1) * QW])
        maps.append(m)
    return maps


LAST_RESULT = {}


def kernel(x, wq, wk, wv, wo, mask=None, trace=False):
    if "nc" not in _CACHE:
        _CACHE["nc"] = _build_nc()
    nc = _CACHE["nc"]
    in_maps = _host_inputs(x, wq, wk, wv, wo)
    res = run_bass_kernel_spmd(nc, in_maps, list(range(NCORES)), trace=trace)
    LAST_RESULT["exec_time_ns"] = res.exec_time_ns
    LAST_RESULT["profile_json"] = res.profile_json
    it = res.instructions_and_trace
    LAST_RESULT["trace_dir"] = it if isinstance(it, str) else None
    full = np.concatenate(
        [res.results[c]["outT"].astype(np.float32).T for c in range(NCORES)],
        axis=1)
    return np.ascontiguousarray(full).reshape(1, S, DIM).astype(np.float32)


# revision 54
# speedup vs baseline: 1.0630x; 1.0630x over previous
"""GQA causal attention block (RoPE, 32 Q heads / 8 KV heads, S=2048, D=4096)
distributed tensor-parallel over heads across 8 TRN2 NeuronCores.

Per core c:
  - 4 query heads (wq cols 512c:512c+512), 1 KV head (wk/wv cols 128c:128c+128)
  - projections computed transposed (qT/kT/vT [hd, seq]) from host-transposed
    xT, weights as stationary operands, bf16 matmuls with f32 PSUM accum
  - RoPE applied with host tables; the half-rotation uses a PE permutation
    matmul (cross-partition moves are impossible on DVE)
  - attention in S^T layout [j, i]; heads processed in pairs with a one-jt
    software skew: the PE computes scores for tile jt while the ACT exp of
    tile jt-1 is consumed by the PV/rowsum matmuls, so the exp latency is
    fully hidden behind double-buffered score banks
  - per-pair O^T accumulates in a single bf16 PSUM bank ([128,2,512]), which
    frees banks for score double-buffering; causal masks are multiplicative
    0/1 DVE ops after the exp; row sums accumulate via col-tiled M=1 matmuls
  - 1/l computed as exp(-ln(l)) on ACT, broadcast across partitions with a
    DRAM-bounce stride-0 DMA, normalization on DVE into an AllGather staging
    tile; one DMA + AllGather per 512-seq group overlaps with compute
  - output projection computed transposed (outT = wo_c^T @ Y^T) from
    SBUF-resident wo, bf16 stores; host casts/transposes each shard back
Host gathers by concatenating the 8 (transposed) column shards.

DMA issue is spread across queues (weights on the GpSimd SWDGE queue, bulk
activations on Sync, stores on Scalar) and batched 4 tiles per descriptor
set so no single sequencer saturates. PSUM banks are laid out as four 2KB
regions (r0-r3) and two 4KB regions (r45, r67) shared across phases.
"""

import numpy as np
import ml_dtypes

import concourse.bass as bass
import concourse.mybir as mybir
import concourse.tile as tile
from concourse.bass_utils import run_bass_kernel_spmd

bf16 = mybir.dt.bfloat16
f32 = mybir.dt.float32

NCORES = 8
S = 2048
DIM = 4096
HD = 128
NH = 32
QH = NH // NCORES          # 4 query heads per core
QW = QH * HD               # 512 wq cols per core
ROPE_BASE = 10000.0
SCALE = float(HD) ** -0.5
NSEQ = S // 512            # 4 seq chunks
KT = DIM // 128            # 32 contraction tiles

_CACHE = {}


def _legalize_waits(nc, allowed_default=1):
    """This walrus build rejects instructions carrying more inline sync waits
    than the opcode template allows (0 for Drain, 1 elsewhere). Spill excess
    waits onto standalone EventSemaphore instructions inserted immediately
    before, on the same engine (engine order preserves semantics)."""
    for f in nc.m.functions:
        for bb in f.blocks:
            out = []
            for ins in bb.instructions:
                tname = type(ins).__name__
                si = getattr(ins, "sync_info", None)
                waits = list(si.on_wait) if (si is not None and si.on_wait) else []
                if tname == "InstEventSemaphore":
                    allowed = len(waits)
                elif tname == "InstDrain":
                    allowed = 0
                else:
                    allowed = allowed_default
                if len(waits) > allowed:
                    spill, keep = waits[allowed:], waits[:allowed]
                    for i, w in enumerate(spill):
                        ev = mybir.InstEventSemaphore(
                            name=f"{ins.name}_wfix{i}",
                            engine=ins.engine, ins=[], outs=[],
                        )
                        ev.sync_info = mybir.SyncInfo(on_wait=[w], on_update=[])
                        out.append(ev)
                    si.on_wait = keep
                out.append(ins)
            bb.instructions[:] = out


def _build_nc():
    nc = bass.Bass(num_devices=NCORES)

    # weights and x arrive host-pre-tiled in SBUF layout [partition, ktile,
    # cols] so every DMA line is 1-8KB contiguous (few fat descriptors)
    xT = nc.declare_dram_parameter("xT", [NSEQ, 128, KT, 512], bf16,
                                   isOutput=False)
    wq = nc.declare_dram_parameter("wq", [128, KT, QW], bf16, isOutput=False)
    wk = nc.declare_dram_parameter("wk", [128, KT, HD], bf16, isOutput=False)
    wv = nc.declare_dram_parameter("wv", [128, KT, HD], bf16, isOutput=False)
    wo = nc.declare_dram_parameter("wo", [128, KT, QW], bf16, isOutput=False)
    cosT = nc.declare_dram_parameter("cosT", [HD, S], f32, isOutput=False)
    sinT = nc.declare_dram_parameter("sinT", [HD, S], f32, isOutput=False)
    maskB = nc.declare_dram_parameter("maskB", [4, 128, 512], bf16, isOutput=False)
    perm = nc.declare_dram_parameter("perm", [128, 128], bf16, isOutput=False)
    ident = nc.declare_dram_parameter("ident", [128, 128], bf16, isOutput=False)
    outT = nc.declare_dram_parameter("outT", [QW, S], bf16, isOutput=True)

    ag_in = nc.dram_tensor("ag_in", [4, QW, 512], bf16)
    ag_outs = [
        nc.dram_tensor(f"ag_out{g}", [NCORES, QW, 512], bf16, addr_space="Shared")
        for g in range(4)
    ]

    maskr = maskB.rearrange("a p m -> p a m")
    agi_r = ag_in.rearrange("g (h p) m -> g p h m", p=128)
    ago_r = [a.rearrange("c (d p) m -> c p d m", p=128) for a in ag_outs]
    outT_r = outT.rearrange("(a p) m -> p a m", p=128)

    with tile.TileContext(nc) as tc:
        with (
            tc.tile_pool(name="const", bufs=1) as constp,
            tc.tile_pool(name="acts", bufs=1) as acts,
            tc.tile_pool(name="xin", bufs=3) as xin,
            tc.tile_pool(name="rope", bufs=2) as rope,
            tc.tile_pool(name="pt", bufs=3) as ptp,
            tc.tile_pool(name="epi", bufs=1) as epi,
            tc.tile_pool(name="cproj", bufs=4) as cproj,
            tc.tile_pool(name="psum", bufs=1, space="PSUM") as psum,
        ):
            # PSUM layout: r0-r3 are 2KB regions (banks 0-3), r45/r67 are 4KB
            # regions (banks 4-5 / 6-7). Shapes/dtypes vary per phase; the
            # byte footprint per tag is constant.
            def preg(tag, shape, dtype=f32, name="ps"):
                return psum.tile(list(shape), dtype, tag=tag,
                                 name=f"{name}_{tag}", bufs=1)

            # small constants (cheap DMAs on sync)
            perm_sb = constp.tile([128, 128], bf16)
            nc.sync.dma_start(perm_sb[:], perm[:])
            ident_sb = constp.tile([128, 128], bf16)
            nc.sync.dma_start(ident_sb[:], ident[:])
            ones_sb = constp.tile([128, 1], bf16)
            nc.vector.memset(ones_sb[:], 1.0)
            ones_row = constp.tile([33, 128], bf16)
            nc.vector.memset(ones_row[:], 1.0)

            # persistent SBUF residents
            wq_sb = constp.tile([128, KT, QW], bf16)
            wk_sb = constp.tile([128, KT, HD], bf16)
            wv_sb = constp.tile([128, KT, HD], bf16)
            wo_sb = constp.tile([128, KT, QW], bf16)
            cos_sb = constp.tile([HD, S], f32)
            sin_sb = constp.tile([HD, S], f32)
            mask_sb = constp.tile([128, 4, 512], bf16)

            # DMA-queue arbitration is near-exhaustive per queue, so a bulk
            # flood on any queue starves the others. Chunk 0's x tiles and wq
            # share the gpsimd queue in exact consumption order (self-paced);
            # wk/wv/cos/sin/mask/wo stream on scalar; sync only carries the
            # later chunks' x tiles plus phase B/C traffic.

            # activations that live through attention
            qTr = acts.tile([128, QH, S], bf16)      # 4 head tiles [hd, seq]
            kTr = acts.tile([128, S], bf16)
            v_sb = acts.tile([128, S], bf16)         # 16 [seq,hd] tiles at jt*128

            # ---- phase A: projections + rope ----
            WQC = {0: (2, 8), 1: (8, 16), 3: (16, 24), 5: (24, 32)}
            for n in range(NSEQ):
                sl = bass.ts(n, 512)
                x4s = []
                for t in range(8):
                    x4 = xin.tile([128, 4, 512], bf16, tag="x")
                    if n == 0:
                        if t == 0:
                            nc.gpsimd.dma_start(wq_sb[:, 0:2], wq[:, 0:2])
                            nc.scalar.dma_start(wk_sb[:, 0:2], wk[:, 0:2])
                            nc.scalar.dma_start(wv_sb[:, 0:2], wv[:, 0:2])
                            nc.gpsimd.dma_start(x4[:, 0:1], xT[n][:, 0:1])
                            nc.gpsimd.dma_start(x4[:, 1:4], xT[n][:, 1:4])
                        else:
                            nc.gpsimd.dma_start(x4[:], xT[n][:, bass.ts(t, 4)])
                        if t in WQC:
                            a, b = WQC[t]
                            nc.gpsimd.dma_start(wq_sb[:, a:b], wq[:, a:b])
                    else:
                        nc.sync.dma_start(x4[:], xT[n][:, bass.ts(t, 4)])
                    x4s.append(x4)
                if n == 0:
                    for i in range(4):
                        lo = slice(8 * i + (2 if i == 0 else 0), 8 * i + 8)
                        nc.scalar.dma_start(wk_sb[:, lo], wk[:, lo])
                        nc.scalar.dma_start(wv_sb[:, lo], wv[:, lo])
                    nc.scalar.dma_start(cos_sb[:], cosT[:])
                    nc.scalar.dma_start(sin_sb[:], sinT[:])
                    nc.scalar.dma_start(mask_sb[:], maskr[:])
                    for i in range(4):
                        nc.scalar.dma_start(wo_sb[:, bass.ts(i, 8)],
                                            wo[:, bass.ts(i, 8)])
                q_ps = [preg(f"r{m}", (128, 512), name="q") for m in range(QH)]
                k_ps = preg("s4", (128, 512), name="k")[:]
                vT_ps = preg("s5", (128, 512), name="vT")[:]
                for k in range(KT):
                    x_sb = x4s[k // 4][:, k % 4]
                    st, sp = (k == 0), (k == KT - 1)
                    for m in range(QH):
                        nc.tensor.matmul(q_ps[m][:], wq_sb[:, k, bass.ts(m, 128)],
                                         x_sb, start=st, stop=sp)
                    nc.tensor.matmul(k_ps, wk_sb[:, k], x_sb, start=st, stop=sp)
                    nc.tensor.matmul(vT_ps, wv_sb[:, k], x_sb, start=st, stop=sp)

                # rope: free the accumulation banks first (ACT copy + DVE
                # cos-mul per output), then the rotation matmuls and combines
                t_bfs, t1s = [], []
                for idx in range(QH + 1):
                    src = q_ps[idx][:] if idx < QH else k_ps
                    t_bf = rope.tile([128, 512], bf16, tag=f"tbf{idx}",
                                     name=f"tbf{idx}", bufs=1)
                    nc.scalar.copy(t_bf[:], src)
                    t1 = rope.tile([128, 512], bf16, tag=f"t1_{idx}",
                                   name=f"t1_{idx}", bufs=1)
                    nc.vector.tensor_mul(t1[:], src, cos_sb[:, sl])
                    t_bfs.append(t_bf)
                    t1s.append(t1)
                for idx in range(QH + 1):
                    dst = qTr[:, idx, sl] if idx < QH else kTr[:, sl]
                    sw_ps = preg("s6" if idx % 2 == 0 else "r7",
                                 (128, 512), name="sw")[:]
                    nc.tensor.matmul(sw_ps, perm_sb[:], t_bfs[idx][:],
                                     start=True, stop=True)
                    t2 = rope.tile([128, 512], bf16, tag=f"t2_{idx % 2}",
                                   name=f"t2_{idx % 2}", bufs=1)
                    nc.vector.tensor_mul(t2[:], sw_ps, sin_sb[:, sl])
                    nc.vector.tensor_add(dst, t1s[idx][:], t2[:])

                # v: copy vT chunk, transpose 128-blocks into [seq, hd] tiles
                v_bf = rope.tile([128, 512], bf16, tag="vbf")
                nc.scalar.copy(v_bf[:], vT_ps)
                for t in range(4):
                    vt_ps = preg("s6" if t % 2 == 0 else "r7",
                                 (128, 128), dtype=bf16, name="vt")
                    nc.tensor.transpose(vt_ps[:], v_bf[:, bass.ts(t, 128)],
                                        ident_sb[:])
                    nc.vector.tensor_copy(out=v_sb[:, bass.ts(4 * n + t, 128)],
                                          in_=vt_ps[:])
                if n == NSEQ - 1:
                    # keep the PE warm through the final rope drain so HAM
                    # doesn't re-throttle before attention starts
                    for dmy in range(6):
                        dps = preg(f"r{dmy % 2}", (128, 512), name="warm")
                        nc.tensor.matmul(dps[:], perm_sb[:], kTr[:, 0:512],
                                         start=True, stop=True)

            # ---- phase B: attention, S^T layout, head pairs, one-jt skew ----
            for g in range(4):
                isl = bass.ts(g, 512)
                njt = 4 * g + 4
                ag_sb = epi.tile([128, QH, 512], bf16, tag="agsb", bufs=2)
                # l for all four heads shares bank r7 (rows 0/32/64/96 via
                # col-group tile_position); matmul start=True would clear the
                # whole bank's has_written bits and corrupt the other rows,
                # so the bank is zeroed once and every l matmul accumulates
                # with start=False (add-to-zero or overwrite, both correct)
                l_ps = preg("r7", (128, 512), name="l")
                nc.vector.memset(l_ps[:], 0.0)
                for p in range(2):
                    oT_ps = [preg(f"r{2 * p + hh}", (128, 512), name="oT")
                             for hh in range(2)]
                    prev = None
                    for jt in range(njt):
                        ksl = bass.ts(jt, 128)
                        pt = ptp.tile([128, 2, 512], bf16, tag="pt")
                        sts = []
                        for hh in range(2):
                            slot = (2 * jt + hh) % 3
                            stp = preg(f"s{4 + slot}", (128, 512), name="st")
                            nc.tensor.matmul(stp[:], kTr[:, ksl],
                                             qTr[:, 2 * p + hh, isl],
                                             start=True, stop=True)
                            sts.append(stp)
                        for hh in range(2):
                            nc.scalar.activation(
                                pt[:, hh], sts[hh][:],
                                mybir.ActivationFunctionType.Exp, scale=SCALE)
                        r = jt - 4 * g
                        if r >= 0:
                            ptm = ptp.tile([128, 2, 512], bf16, tag="ptm")
                            for hh in range(2):
                                nc.vector.tensor_mul(ptm[:, hh], pt[:, hh],
                                                     mask_sb[:, r])
                            pt = ptm

                        def consume(cjt, cpt):
                            cksl = bass.ts(cjt, 128)
                            for hh in range(2):
                                nc.tensor.matmul(oT_ps[hh][:], v_sb[:, cksl],
                                                 cpt[:, hh], start=(cjt == 0),
                                                 stop=(cjt == njt - 1))
                            for hh in range(2):
                                row = 64 * p + 32 * hh
                                nc.tensor.matmul(
                                    l_ps[bass.ds(row, 1), :], ones_sb[:],
                                    cpt[:, hh], start=False,
                                    stop=(cjt == njt - 1),
                                    tile_position=(0, row),
                                    skip_group_check=True)

                        if prev is not None:
                            consume(*prev)
                        prev = (jt, pt)
                    consume(*prev)
                    # epilogue: 1/l = exp(-ln(l)) on ACT, partition-broadcast
                    # via DRAM-bounce stride-0 DMA, normalize on DVE
                    ln_sb = epi.tile([33, 512], f32, tag=f"ln{p}")
                    nc.scalar.activation(ln_sb[:], l_ps[64 * p:64 * p + 33, :],
                                         mybir.ActivationFunctionType.Ln)
                    linv_sb = epi.tile([33, 512], bf16, tag=f"linv{p}")
                    nc.scalar.activation(linv_sb[:], ln_sb[:],
                                         mybir.ActivationFunctionType.Exp,
                                         scale=-1.0)
                    # partition-broadcast 1/l rows with a K=1 matmul
                    # (ones_row^T @ linv_row), DVE-copy to SBUF, normalize
                    for hh in range(2):
                        lb_ps = preg("s6", (128, 512), name="lb")[:]
                        nc.tensor.matmul(lb_ps, ones_row[bass.ds(32 * hh, 1), :],
                                         linv_sb[bass.ds(32 * hh, 1), :],
                                         start=True, stop=True)
                        lb_sb = epi.tile([128, 512], f32, tag=f"lb{hh}")
                        nc.vector.tensor_copy(out=lb_sb[:], in_=lb_ps)
                        nc.vector.tensor_mul(ag_sb[:, 2 * p + hh],
                                             oT_ps[hh][:], lb_sb[:])
                nc.sync.dma_start(agi_r[g], ag_sb[:])
                nc.gpsimd.collective_compute(
                    "AllGather", mybir.AluOpType.bypass,
                    replica_groups=[list(range(NCORES))],
                    ins=[ag_in[g]], outs=[ag_outs[g][:]],
                )

            # ---- phase C: outT = wo_c^T @ Y^T, wo stationary from SBUF ----
            for ns in range(NSEQ):
                nsl = bass.ts(ns, 512)
                y4s = []
                for c in range(NCORES):
                    y4 = cproj.tile([128, 4, 512], bf16, tag="y")
                    nc.sync.dma_start(y4[:], ago_r[ns][c])
                    y4s.append(y4)
                # even ns uses banks 4-7 (free right after phase B's scores),
                # odd ns uses banks 0-3 (freed by the last epilogue muls)
                if ns % 2 == 0:
                    o_ps = [preg(t, (128, 512), name="o")[:]
                            for t in ("s4", "s5", "s6", "r7")]
                else:
                    o_ps = [preg(f"r{ob}", (128, 512), name="o")[:]
                            for ob in range(QH)]
                for kt in range(KT):
                    y_sb = y4s[kt // 4][:, kt % 4]
                    for ob in range(QH):
                        nc.tensor.matmul(
                            o_ps[ob], wo_sb[:, kt, bass.ts(ob, 128)], y_sb,
                            start=(kt == 0), stop=(kt == KT - 1))
                o_stage = cproj.tile([128, 4, 512], bf16, tag="ostg", bufs=2)
                for ob in range(QH):
                    nc.scalar.copy(o_stage[:, ob], o_ps[ob])
                    if ob % 2 == 1:
                        nc.scalar.dma_start(
                            outT_r[:, ob - 1:ob + 1, nsl],
                            o_stage[:, ob - 1:ob + 1])

    _legalize_waits(nc)
    return nc


def _pretile(w):
    """[DIM, C] -> SBUF layout [128, KT, C] (partition-major k-tiles)."""
    c = w.shape[1]
    return np.ascontiguousarray(
        np.asarray(w).reshape(KT, 128, c).transpose(1, 0, 2)
    ).astype(ml_dtypes.bfloat16)


def _host_inputs(x, wq, wk, wv, wo):
    x = np.asarray(x, dtype=np.float32)
    xT = np.ascontiguousarray(x.reshape(S, DIM).T)          # [DIM, S]
    # pre-tile x per 512-seq chunk: xP[n, p, a, m] = xT[a*128+p, n*512+m]
    xP = np.ascontiguousarray(
        xT.reshape(KT, 128, NSEQ, 512).transpose(2, 1, 0, 3)
    ).astype(ml_dtypes.bfloat16)

    # rope tables in [hd, seq] layout with the sign of sin baked in
    inv_freq = 1.0 / ROPE_BASE ** (np.arange(0, HD, 2, dtype=np.float32) / HD)
    t = np.arange(S, dtype=np.float32)
    freqs = np.outer(inv_freq, t)                       # [64, S]
    cosT = np.concatenate([np.cos(freqs), np.cos(freqs)], 0).astype(np.float32)
    sinT = np.concatenate([-np.sin(freqs), np.sin(freqs)], 0).astype(np.float32)

    # S^T-layout multiplicative diagonal masks:
    # maskB[r][j, i] = 1 if r*128 + j <= i else 0
    j = np.arange(128)[None, :, None]
    i = np.arange(512)[None, None, :]
    r = np.arange(4)[:, None, None]
    maskB = np.where(r * 128 + j <= i, 1.0, 0.0).astype(ml_dtypes.bfloat16)

    perm = np.zeros((128, 128), dtype=np.float32)
    perm[np.arange(128), (np.arange(128) + 64) % 128] = 1.0
    ident = np.eye(128, dtype=np.float32)

    shared = {
        "xT": xP,
        "cosT": cosT,
        "sinT": sinT,
        "maskB": maskB,
        "perm": perm.astype(ml_dtypes.bfloat16),
        "ident": ident.astype(ml_dtypes.bfloat16),
    }
    maps = []
    for c in range(NCORES):
        m = dict(shared)
        m["wq"] = _pretile(wq[:, c * QW:(c + 1) * QW])
        m["wk"] = _pretile(wk[:, c * HD:(c + 1) * HD])
        m["wv"] = _pretile(wv[:, c * HD:(c + 1) * HD])
        m["wo"] = _pretile(wo[:, c * QW:(c + 1) * QW])
        maps.append(m)
    return maps


LAST_RESULT = {}


def kernel(x, wq, wk, wv, wo, mask=None, trace=False):
    if "nc" not in _CACHE:
        _CACHE["nc"] = _build_nc()
    nc = _CACHE["nc"]
    in_maps = _host_inputs(x, wq, wk, wv, wo)
    res = run_bass_kernel_spmd(nc, in_maps, list(range(NCORES)), trace=trace)
    LAST_RESULT["exec_time_ns"] = res.exec_time_ns
    LAST_RESULT["profile_json"] = res.profile_json
    it = res.instructions_and_trace
    LAST_RESULT["trace_dir"] = it if isinstance(it, str) else None
    full = np.concatenate(
        [res.results[c]["outT"].astype(np.float32).T for c in range(NCORES)],
        axis=1)
    return np.ascontiguousarray(full).reshape(1, S, DIM).astype(np.float32)


# revision 55
# speedup vs baseline: 1.1139x; 1.0478x over previous
"""GQA causal attention block (RoPE, 32 Q heads / 8 KV heads, S=2048, D=4096)
distributed tensor-parallel over heads across 8 TRN2 NeuronCores.

Per core c:
  - 4 query heads (wq cols 512c:512c+512), 1 KV head (wk/wv cols 128c:128c+128)
  - projections computed transposed (qT/kT/vT [hd, seq]) from host-transposed
    xT, weights as stationary operands, bf16 matmuls with f32 PSUM accum
  - RoPE applied with host tables; the half-rotation uses a PE permutation
    matmul (cross-partition moves are impossible on DVE)
  - attention in S^T layout [j, i]; heads processed in pairs with a one-jt
    software skew: the PE computes scores for tile jt while the ACT exp of
    tile jt-1 is consumed by the PV/rowsum matmuls, so the exp latency is
    fully hidden behind double-buffered score banks
  - per-pair O^T accumulates in a single bf16 PSUM bank ([128,2,512]), which
    frees banks for score double-buffering; causal masks are multiplicative
    0/1 DVE ops after the exp; row sums accumulate via col-tiled M=1 matmuls
  - 1/l computed as exp(-ln(l)) on ACT, broadcast across partitions with a
    DRAM-bounce stride-0 DMA, normalization on DVE into an AllGather staging
    tile; one DMA + AllGather per 512-seq group overlaps with compute
  - output projection computed transposed (outT = wo_c^T @ Y^T) from
    SBUF-resident wo, bf16 stores; host casts/transposes each shard back
Host gathers by concatenating the 8 (transposed) column shards.

DMA issue is spread across queues (weights on the GpSimd SWDGE queue, bulk
activations on Sync, stores on Scalar) and batched 4 tiles per descriptor
set so no single sequencer saturates. PSUM banks are laid out as four 2KB
regions (r0-r3) and two 4KB regions (r45, r67) shared across phases.
"""

import numpy as np
import ml_dtypes

import concourse.bass as bass
import concourse.mybir as mybir
import concourse.tile as tile
from concourse.bass_utils import run_bass_kernel_spmd

bf16 = mybir.dt.bfloat16
f32 = mybir.dt.float32

NCORES = 8
S = 2048
DIM = 4096
HD = 128
NH = 32
QH = NH // NCORES          # 4 query heads per core
QW = QH * HD               # 512 wq cols per core
ROPE_BASE = 10000.0
SCALE = float(HD) ** -0.5
NSEQ = S // 512            # 4 seq chunks
KT = DIM // 128            # 32 contraction tiles

_CACHE = {}


def _legalize_waits(nc, allowed_default=1):
    """This walrus build rejects instructions carrying more inline sync waits
    than the opcode template allows (0 for Drain, 1 elsewhere). Spill excess
    waits onto standalone EventSemaphore instructions inserted immediately
    before, on the same engine (engine order preserves semantics)."""
    for f in nc.m.functions:
        for bb in f.blocks:
            out = []
            for ins in bb.instructions:
                tname = type(ins).__name__
                si = getattr(ins, "sync_info", None)
                waits = list(si.on_wait) if (si is not None and si.on_wait) else []
                if tname == "InstEventSemaphore":
                    allowed = len(waits)
                elif tname == "InstDrain":
                    allowed = 0
                else:
                    allowed = allowed_default
                if len(waits) > allowed:
                    spill, keep = waits[allowed:], waits[:allowed]
                    for i, w in enumerate(spill):
                        ev = mybir.InstEventSemaphore(
                            name=f"{ins.name}_wfix{i}",
                            engine=ins.engine, ins=[], outs=[],
                        )
                        ev.sync_info = mybir.SyncInfo(on_wait=[w], on_update=[])
                        out.append(ev)
                    si.on_wait = keep
                out.append(ins)
            bb.instructions[:] = out


def _build_nc():
    nc = bass.Bass(num_devices=NCORES)

    # weights and x arrive host-pre-tiled in SBUF layout [partition, ktile,
    # cols] so every DMA line is 1-8KB contiguous (few fat descriptors)
    xT = nc.declare_dram_parameter("xT", [NSEQ, 128, KT, 512], bf16,
                                   isOutput=False)
    wq = nc.declare_dram_parameter("wq", [128, KT, QW], bf16, isOutput=False)
    wk = nc.declare_dram_parameter("wk", [128, KT, HD], bf16, isOutput=False)
    wv = nc.declare_dram_parameter("wv", [128, KT, HD], bf16, isOutput=False)
    wo = nc.declare_dram_parameter("wo", [128, KT, QW], bf16, isOutput=False)
    cosT = nc.declare_dram_parameter("cosT", [HD, S], f32, isOutput=False)
    sinT = nc.declare_dram_parameter("sinT", [HD, S], f32, isOutput=False)
    maskB = nc.declare_dram_parameter("maskB", [4, 128, 512], bf16, isOutput=False)
    perm = nc.declare_dram_parameter("perm", [128, 128], bf16, isOutput=False)
    ident = nc.declare_dram_parameter("ident", [128, 128], bf16, isOutput=False)
    outT = nc.declare_dram_parameter("outT", [QW, S], bf16, isOutput=True)

    ag_in = nc.dram_tensor("ag_in", [4, QW, 512], bf16)
    ag_outs = [
        nc.dram_tensor(f"ag_out{g}", [NCORES, QW, 512], bf16, addr_space="Shared")
        for g in range(4)
    ]

    maskr = maskB.rearrange("a p m -> p a m")
    agi_r = ag_in.rearrange("g (h p) m -> g p h m", p=128)
    ago_r = [a.rearrange("c (d p) m -> c p d m", p=128) for a in ag_outs]
    outT_r = outT.rearrange("(a p) m -> p a m", p=128)

    with tile.TileContext(nc) as tc:
        with (
            tc.tile_pool(name="const", bufs=1) as constp,
            tc.tile_pool(name="acts", bufs=1) as acts,
            tc.tile_pool(name="xin", bufs=3) as xin,
            tc.tile_pool(name="rope", bufs=2) as rope,
            tc.tile_pool(name="pt", bufs=3) as ptp,
            tc.tile_pool(name="epi", bufs=1) as epi,
            tc.tile_pool(name="cproj", bufs=4) as cproj,
            tc.tile_pool(name="psum", bufs=1, space="PSUM") as psum,
        ):
            # PSUM layout: r0-r3 are 2KB regions (banks 0-3), r45/r67 are 4KB
            # regions (banks 4-5 / 6-7). Shapes/dtypes vary per phase; the
            # byte footprint per tag is constant.
            def preg(tag, shape, dtype=f32, name="ps"):
                return psum.tile(list(shape), dtype, tag=tag,
                                 name=f"{name}_{tag}", bufs=1)

            # small constants (cheap DMAs on sync)
            perm_sb = constp.tile([128, 128], bf16)
            nc.sync.dma_start(perm_sb[:], perm[:])
            ident_sb = constp.tile([128, 128], bf16)
            nc.sync.dma_start(ident_sb[:], ident[:])
            ones_sb = constp.tile([128, 1], bf16)
            nc.vector.memset(ones_sb[:], 1.0)
            ones_row = constp.tile([33, 128], bf16)
            nc.vector.memset(ones_row[:], 1.0)

            # persistent SBUF residents
            wq_sb = constp.tile([128, KT, QW], bf16)
            wk_sb = constp.tile([128, KT, HD], bf16)
            wv_sb = constp.tile([128, KT, HD], bf16)
            wo_sb = constp.tile([128, KT, QW], bf16)
            cos_sb = constp.tile([HD, S], f32)
            sin_sb = constp.tile([HD, S], f32)
            mask_sb = constp.tile([128, 4, 512], bf16)

            # DMA-queue arbitration is near-exhaustive per queue, so a bulk
            # flood on any queue starves the others. Chunk 0's x tiles and wq
            # share the gpsimd queue in exact consumption order (self-paced);
            # wk/wv/cos/sin/mask/wo stream on scalar; sync only carries the
            # later chunks' x tiles plus phase B/C traffic.

            # activations that live through attention
            qTr = acts.tile([128, QH, S], bf16)      # 4 head tiles [hd, seq]
            kTr = acts.tile([128, S], bf16)
            v_sb = acts.tile([128, S], bf16)         # 16 [seq,hd] tiles at jt*128

            # ---- phase A: projections + rope ----
            WQC = {0: (2, 8), 1: (8, 16), 3: (16, 24), 5: (24, 32)}
            for n in range(NSEQ):
                sl = bass.ts(n, 512)
                x4s = []
                for t in range(8):
                    x4 = xin.tile([128, 4, 512], bf16, tag="x")
                    if n == 0:
                        if t == 0:
                            nc.gpsimd.dma_start(wq_sb[:, 0:2], wq[:, 0:2])
                            nc.scalar.dma_start(wk_sb[:, 0:2], wk[:, 0:2])
                            nc.scalar.dma_start(wv_sb[:, 0:2], wv[:, 0:2])
                            nc.gpsimd.dma_start(x4[:, 0:1], xT[n][:, 0:1])
                            nc.gpsimd.dma_start(x4[:, 1:4], xT[n][:, 1:4])
                        else:
                            nc.gpsimd.dma_start(x4[:], xT[n][:, bass.ts(t, 4)])
                        if t in WQC:
                            a, b = WQC[t]
                            nc.gpsimd.dma_start(wq_sb[:, a:b], wq[:, a:b])
                    else:
                        nc.sync.dma_start(x4[:], xT[n][:, bass.ts(t, 4)])
                    x4s.append(x4)
                if n == 0:
                    for i in range(4):
                        lo = slice(8 * i + (2 if i == 0 else 0), 8 * i + 8)
                        nc.scalar.dma_start(wk_sb[:, lo], wk[:, lo])
                        nc.scalar.dma_start(wv_sb[:, lo], wv[:, lo])
                    nc.scalar.dma_start(cos_sb[:], cosT[:])
                    nc.scalar.dma_start(sin_sb[:], sinT[:])
                    nc.scalar.dma_start(mask_sb[:], maskr[:])
                    for i in range(4):
                        nc.scalar.dma_start(wo_sb[:, bass.ts(i, 8)],
                                            wo[:, bass.ts(i, 8)])
                q_ps = [preg(f"r{m}", (128, 512), name="q") for m in range(QH)]
                k_ps = preg("s4", (128, 512), name="k")[:]
                vT_ps = preg("s5", (128, 512), name="vT")[:]
                for k in range(KT):
                    x_sb = x4s[k // 4][:, k % 4]
                    st, sp = (k == 0), (k == KT - 1)
                    for m in range(QH):
                        nc.tensor.matmul(q_ps[m][:], wq_sb[:, k, bass.ts(m, 128)],
                                         x_sb, start=st, stop=sp)
                    nc.tensor.matmul(k_ps, wk_sb[:, k], x_sb, start=st, stop=sp)
                    nc.tensor.matmul(vT_ps, wv_sb[:, k], x_sb, start=st, stop=sp)

                # rope: free the accumulation banks first (ACT copy + DVE
                # cos-mul per output), then the rotation matmuls and combines
                t_bfs, t1s = [], []
                for idx in range(QH + 1):
                    src = q_ps[idx][:] if idx < QH else k_ps
                    t_bf = rope.tile([128, 512], bf16, tag=f"tbf{idx}",
                                     name=f"tbf{idx}", bufs=1)
                    nc.scalar.copy(t_bf[:], src)
                    t1 = rope.tile([128, 512], bf16, tag=f"t1_{idx}",
                                   name=f"t1_{idx}", bufs=1)
                    nc.vector.tensor_mul(t1[:], src, cos_sb[:, sl])
                    t_bfs.append(t_bf)
                    t1s.append(t1)
                for idx in range(QH + 1):
                    dst = qTr[:, idx, sl] if idx < QH else kTr[:, sl]
                    sw_ps = preg("s6" if idx % 2 == 0 else "r7",
                                 (128, 512), name="sw")[:]
                    nc.tensor.matmul(sw_ps, perm_sb[:], t_bfs[idx][:],
                                     start=True, stop=True)
                    t2 = rope.tile([128, 512], bf16, tag=f"t2_{idx % 2}",
                                   name=f"t2_{idx % 2}", bufs=1)
                    nc.vector.tensor_mul(t2[:], sw_ps, sin_sb[:, sl])
                    nc.vector.tensor_add(dst, t1s[idx][:], t2[:])

                # v: copy vT chunk, transpose 128-blocks into [seq, hd] tiles
                v_bf = rope.tile([128, 512], bf16, tag="vbf")
                nc.scalar.copy(v_bf[:], vT_ps)
                for t in range(4):
                    vt_ps = preg("s6" if t % 2 == 0 else "r7",
                                 (128, 128), dtype=bf16, name="vt")
                    nc.tensor.transpose(vt_ps[:], v_bf[:, bass.ts(t, 128)],
                                        ident_sb[:])
                    nc.vector.tensor_copy(out=v_sb[:, bass.ts(4 * n + t, 128)],
                                          in_=vt_ps[:])
                if n == NSEQ - 1:
                    # keep the PE warm through the final rope drain so HAM
                    # doesn't re-throttle before attention starts
                    for dmy in range(6):
                        dps = preg(f"r{dmy % 2}", (128, 512), name="warm")
                        nc.tensor.matmul(dps[:], perm_sb[:], kTr[:, 0:512],
                                         start=True, stop=True)

            # ---- phase B: attention, S^T layout, head pairs, one-jt skew ----
            # Each pass's softmax epilogue is DEFERRED and emitted just after
            # the next pass's first exp, so the next pass's exps aren't
            # queued behind Ln/Exp on the in-order ACT sequencer (removes a
            # ~1.6us PE stall per pass boundary).
            pending_epi = [None]

            def flush_epi():
                if pending_epi[0] is not None:
                    pending_epi[0]()
                    pending_epi[0] = None

            def make_epi(g, p, oT_ps, l_ps, ag_sb, is_last_pair):
                def emit():
                    # 1/l = exp(-ln(l)) on ACT, partition-broadcast with a
                    # K=1 matmul (ones_row^T @ linv_row), DVE-copy, normalize
                    ln_sb = epi.tile([33, 512], f32, tag=f"ln{p}")
                    nc.scalar.activation(ln_sb[:], l_ps[64 * p:64 * p + 33, :],
                                         mybir.ActivationFunctionType.Ln)
                    linv_sb = epi.tile([33, 512], bf16, tag=f"linv{p}")
                    nc.scalar.activation(linv_sb[:], ln_sb[:],
                                         mybir.ActivationFunctionType.Exp,
                                         scale=-1.0)
                    for hh in range(2):
                        lb_ps = preg("s6", (128, 512), name="lb")[:]
                        nc.tensor.matmul(lb_ps,
                                         ones_row[bass.ds(32 * hh, 1), :],
                                         linv_sb[bass.ds(32 * hh, 1), :],
                                         start=True, stop=True)
                        lb_sb = epi.tile([128, 512], f32, tag=f"lb{hh}")
                        nc.vector.tensor_copy(out=lb_sb[:], in_=lb_ps)
                        nc.vector.tensor_mul(ag_sb[:, 2 * p + hh],
                                             oT_ps[hh][:], lb_sb[:])
                    if is_last_pair:
                        nc.sync.dma_start(agi_r[g], ag_sb[:])
                        nc.gpsimd.collective_compute(
                            "AllGather", mybir.AluOpType.bypass,
                            replica_groups=[list(range(NCORES))],
                            ins=[ag_in[g]], outs=[ag_outs[g][:]],
                        )
                return emit

            for g in range(4):
                isl = bass.ts(g, 512)
                njt = 4 * g + 4
                ag_sb = epi.tile([128, QH, 512], bf16, tag="agsb", bufs=2)
                # l for all four heads shares bank r7 (rows 0/32/64/96 via
                # col-group tile_position); matmul start=True would clear the
                # whole bank's has_written bits and corrupt the other rows,
                # so the bank is zeroed once and every l matmul accumulates
                # with start=False (add-to-zero or overwrite, both correct)
                l_ps = preg("r7", (128, 512), name="l")
                for p in range(2):
                    oT_ps = [preg(f"r{2 * p + hh}", (128, 512), name="oT")
                             for hh in range(2)]
                    prev = None
                    for jt in range(njt):
                        ksl = bass.ts(jt, 128)
                        pt = ptp.tile([128, 2, 512], bf16, tag="pt")
                        sts = []
                        for hh in range(2):
                            slot = (2 * jt + hh) % 3
                            stp = preg(f"s{4 + slot}", (128, 512), name="st")
                            nc.tensor.matmul(stp[:], kTr[:, ksl],
                                             qTr[:, 2 * p + hh, isl],
                                             start=True, stop=True)
                            sts.append(stp)
                        for hh in range(2):
                            nc.scalar.activation(
                                pt[:, hh], sts[hh][:],
                                mybir.ActivationFunctionType.Exp, scale=SCALE)
                        if jt == 0:
                            # prior pass's epilogue lands here, behind this
                            # pass's first exp in the ACT queue
                            flush_epi()
                            if p == 0:
                                nc.vector.memset(l_ps[:], 0.0)
                        r = jt - 4 * g
                        if r >= 0:
                            ptm = ptp.tile([128, 2, 512], bf16, tag="ptm")
                            for hh in range(2):
                                nc.vector.tensor_mul(ptm[:, hh], pt[:, hh],
                                                     mask_sb[:, r])
                            pt = ptm

                        def consume(cjt, cpt):
                            cksl = bass.ts(cjt, 128)
                            for hh in range(2):
                                nc.tensor.matmul(oT_ps[hh][:], v_sb[:, cksl],
                                                 cpt[:, hh], start=(cjt == 0),
                                                 stop=(cjt == njt - 1))
                            for hh in range(2):
                                row = 64 * p + 32 * hh
                                nc.tensor.matmul(
                                    l_ps[bass.ds(row, 1), :], ones_sb[:],
                                    cpt[:, hh], start=False,
                                    stop=(cjt == njt - 1),
                                    tile_position=(0, row),
                                    skip_group_check=True)

                        if prev is not None:
                            consume(*prev)
                        prev = (jt, pt)
                    consume(*prev)
                    pending_epi[0] = make_epi(g, p, oT_ps, l_ps, ag_sb,
                                              p == 1)
            flush_epi()

            # ---- phase C: outT = wo_c^T @ Y^T, wo stationary from SBUF ----
            for ns in range(NSEQ):
                nsl = bass.ts(ns, 512)
                y4s = []
                for c in range(NCORES):
                    y4 = cproj.tile([128, 4, 512], bf16, tag="y")
                    nc.sync.dma_start(y4[:], ago_r[ns][c])
                    y4s.append(y4)
                # even ns uses banks 4-7 (free right after phase B's scores),
                # odd ns uses banks 0-3 (freed by the last epilogue muls)
                if ns % 2 == 0:
                    o_ps = [preg(t, (128, 512), name="o")[:]
                            for t in ("s4", "s5", "s6", "r7")]
                else:
                    o_ps = [preg(f"r{ob}", (128, 512), name="o")[:]
                            for ob in range(QH)]
                for kt in range(KT):
                    y_sb = y4s[kt // 4][:, kt % 4]
                    for ob in range(QH):
                        nc.tensor.matmul(
                            o_ps[ob], wo_sb[:, kt, bass.ts(ob, 128)], y_sb,
                            start=(kt == 0), stop=(kt == KT - 1))
                o_stage = cproj.tile([128, 4, 512], bf16, tag="ostg", bufs=2)
                for ob in range(QH):
                    nc.scalar.copy(o_stage[:, ob], o_ps[ob])
                    if ob % 2 == 1:
                        nc.scalar.dma_start(
                            outT_r[:, ob - 1:ob + 1, nsl],
                            o_stage[:, ob - 1:ob + 1])

    _legalize_waits(nc)
    return nc


def _pretile(w):
    """[DIM, C] -> SBUF layout [128, KT, C] (partition-major k-tiles)."""
    c = w.shape[1]
    return np.ascontiguousarray(
        np.asarray(w).reshape(KT, 128, c).transpose(1, 0, 2)
    ).astype(ml_dtypes.bfloat16)


def _host_inputs(x, wq, wk, wv, wo):
    x = np.asarray(x, dtype=np.float32)
    xT = np.ascontiguousarray(x.reshape(S, DIM).T)          # [DIM, S]
    # pre-tile x per 512-seq chunk: xP[n, p, a, m] = xT[a*128+p, n*512+m]
    xP = np.ascontiguousarray(
        xT.reshape(KT, 128, NSEQ, 512).transpose(2, 1, 0, 3)
    ).astype(ml_dtypes.bfloat16)

    # rope tables in [hd, seq] layout with the sign of sin baked in
    inv_freq = 1.0 / ROPE_BASE ** (np.arange(0, HD, 2, dtype=np.float32) / HD)
    t = np.arange(S, dtype=np.float32)
    freqs = np.outer(inv_freq, t)                       # [64, S]
    cosT = np.concatenate([np.cos(freqs), np.cos(freqs)], 0).astype(np.float32)
    sinT = np.concatenate([-np.sin(freqs), np.sin(freqs)], 0).astype(np.float32)

    # S^T-layout multiplicative diagonal masks:
    # maskB[r][j, i] = 1 if r*128 + j <= i else 0
    j = np.arange(128)[None, :, None]
    i = np.arange(512)[None, None, :]
    r = np.arange(4)[:, None, None]
    maskB = np.where(r * 128 + j <= i, 1.0, 0.0).astype(ml_dtypes.bfloat16)

    perm = np.zeros((128, 128), dtype=np.float32)
    perm[np.arange(128), (np.arange(128) + 64) % 128] = 1.0
    ident = np.eye(128, dtype=np.float32)

    shared = {
        "xT": xP,
        "cosT": cosT,
        "sinT": sinT,
        "maskB": maskB,
        "perm": perm.astype(ml_dtypes.bfloat16),
        "ident": ident.astype(ml_dtypes.bfloat16),
    }
    maps = []
    for c in range(NCORES):
        m = dict(shared)
        m["wq"] = _pretile(wq[:, c * QW:(c + 1) * QW])
        m["wk"] = _pretile(wk[:, c * HD:(c + 1) * HD])
        m["wv"] = _pretile(wv[:, c * HD:(c + 1) * HD])
        m["wo"] = _pretile(wo[:, c * QW:(c + 1) * QW])
        maps.append(m)
    return maps


LAST_RESULT = {}


def kernel(x, wq, wk, wv, wo, mask=None, trace=False):
    if "nc" not in _CACHE:
        _CACHE["nc"] = _build_nc()
    nc = _CACHE["nc"]
    in_maps = _host_inputs(x, wq, wk, wv, wo)
    res = run_bass_kernel_spmd(nc, in_maps, list(range(NCORES)), trace=trace)
    LAST_RESULT["exec_time_ns"] = res.exec_time_ns
    LAST_RESULT["profile_json"] = res.profile_json
    it = res.instructions_and_trace
    LAST_RESULT["trace_dir"] = it if isinstance(it, str) else None
    full = np.concatenate(
        [res.results[c]["outT"].astype(np.float32).T for c in range(NCORES)],
        axis=1)
    return np.ascontiguousarray(full).reshape(1, S, DIM).astype(np.float32)
